# revision 8
# baseline (speedup 1.0000x reference)
"""Trainium2 Bass kernel for bipartite GNN metapath aggregation (LATTE).

Reference math:
    h_a = relu(x_a @ W_a + b_a); h_b = relu(x_b @ W_b + b_b)
    A[r,c] = #edges_ab(r,c); B[r,c] = #edges_ba(r,c)
    deg = colcount(A) + rowcount(B); d = 1/deg (0 where deg==0)
    out = (A*d) @ B @ h_a + A @ h_b
Reassociated (exact up to fp reassociation):
    out = A @ (d[:,None] * (B @ h_a) + h_b)

Design G (fully sharded, 2 AllGathers, fp8 adjacency):
  - Nodes sharded 512/core on every step; adjacency stored as fp8e4
    (edge counts are small integers -> exact) and fed directly to the PE
    as the moving operand against bf16 features (mixed-dtype matmul).
  - proj_a sharded; h_a shard transposed to node-major [128,4,128] tiles
    and AllGathered (bypass, bf16). Pre-tiled layout keeps every DMA in
    >=1KB contiguous bursts.
  - SpMM1: t^T = (B_sh @ h_a)^T accumulated feature-major over 32 k-tiles.
  - v^T = d*t^T + h_b^T; transposed to node-major and AllGathered.
  - SpMM2 row-parallel: out_sh^T = v^T-weighted sum over A^T columns;
    written feature-major, host transposes. No ReduceScatter.
"""

import numpy as np
import ml_dtypes

NA = 4096
NB = 4096
FA = 512
FB = 512
D = 128
M = 8          # cores
S = NA // M    # 512 rows per core
KT = NA // 128  # 32 k-tiles over the full node dim
KS = S // 128   # 4 k-tiles over a shard
P = 128

_BUILT = {}


def _emit_iter(nc, tc, pools, tensors, idents):
    import concourse.mybir as mybir

    f32 = mybir.dt.float32
    bf16 = mybir.dt.bfloat16
    fp8 = mybir.dt.float8e4
    Relu = mybir.ActivationFunctionType.Relu
    MUL = mybir.AluOpType.mult

    (xa, wa, xb, wb, bias, drep, bt, at, outT) = tensors
    (ident_bf,) = idents
    bigp, inp, workp, psA, psB, psT, psO, psTr, psTrV, dramp = pools

    # ---- input DMAs ------------------------------------------------------
    xa_sb = inp.tile([P, 4, FA], bf16, tag="xa")
    wa_sb = inp.tile([P, 4, D], bf16, tag="wa")
    xb_sb = inp.tile([P, 4, FB], bf16, tag="xb")
    wb_sb = inp.tile([P, 4, D], bf16, tag="wb")
    bias_sb = inp.tile([P, 2], f32, tag="bias")
    drep_sb = inp.tile([P, S], bf16, tag="drep")
    bt_sb = bigp.tile([P, KT, S], fp8, tag="bt")
    at_sb = bigp.tile([P, KT, S], fp8, tag="at")

    nc.sync.dma_start(out=xa_sb[:], in_=xa[:])
    nc.sync.dma_start(out=wa_sb[:], in_=wa[:])
    nc.sync.dma_start(out=bias_sb[:], in_=bias[:])
    nc.sync.dma_start(out=xb_sb[:], in_=xb[:])
    nc.sync.dma_start(out=wb_sb[:], in_=wb[:])
    nc.sync.dma_start(out=drep_sb[:], in_=drep[:])
    for g in range(4):
        nc.sync.dma_start(
            out=bt_sb[:, g * 8 : (g + 1) * 8, :], in_=bt[:, g * 8 : (g + 1) * 8, :]
        )
    for g in range(4):
        nc.sync.dma_start(
            out=at_sb[:, g * 8 : (g + 1) * 8, :], in_=at[:, g * 8 : (g + 1) * 8, :]
        )

    # ---- proj_a (shard): h_a^T = relu(W_a^T @ x_a^T + b_a), bf16 ---------
    ps_a = psA.tile([P, S], f32, tag="pa")
    for k in range(4):
        nc.tensor.matmul(
            ps_a[:],
            lhsT=wa_sb[:, k, :],
            rhs=xa_sb[:, k, :],
            start=(k == 0),
            stop=(k == 3),
        )
    haT_sb = workp.tile([P, S], bf16, tag="haT")
    nc.scalar.activation(haT_sb[:], ps_a[:], Relu, bias=bias_sb[:, 0:1])

    # ---- transpose h_a^T -> node-major [128, 4, 128] bf16 ----------------
    ha_blk = workp.tile([P, KS, D], bf16, tag="hablk")
    for b in range(KS):
        tp = psTr.tile([P, P], bf16, tag="tr")
        nc.tensor.transpose(
            out=tp[:], in_=haT_sb[:, b * 128 : (b + 1) * 128], identity=ident_bf[:]
        )
        nc.vector.tensor_copy(ha_blk[:, b, :], tp[:])

    # ---- AllGather h_a ---------------------------------------------------
    agin1 = dramp.tile([P, KS, D], bf16, tag="agin1")
    agout1 = dramp.tile([M * P, KS, D], bf16, tag="agout1", addr_space="Shared")
    nc.sync.dma_start(out=agin1[:], in_=ha_blk[:])
    nc.gpsimd.collective_compute(
        "AllGather",
        nc_alu_bypass(),
        replica_groups=[list(range(M))],
        ins=[agin1[:].opt()],
        outs=[agout1[:].opt()],
    )
    ha_node = bigp.tile([P, KT, D], bf16, tag="han")
    for m in range(M):
        nc.sync.dma_start(
            out=ha_node[:, m * KS : (m + 1) * KS, :],
            in_=agout1[m * P : (m + 1) * P, :, :],
        )

    # ---- proj_b (shard): h_b^T = relu(W_b^T @ x_b^T + b_b), f32 ----------
    ps_b = psB.tile([P, S], f32, tag="pb")
    for k in range(4):
        nc.tensor.matmul(
            ps_b[:],
            lhsT=wb_sb[:, k, :],
            rhs=xb_sb[:, k, :],
            start=(k == 0),
            stop=(k == 3),
        )
    hbT_sb = workp.tile([P, S], f32, tag="hbT")
    nc.scalar.activation(hbT_sb[:], ps_b[:], Relu, bias=bias_sb[:, 1:2])

    # ---- SpMM1: t^T = (B_sh @ h_a)^T  [D, S] -----------------------------
    ps_t = psT.tile([P, S], f32, tag="pt")
    for k in range(KT):
        nc.tensor.matmul(
            ps_t[:],
            lhsT=ha_node[:, k, :],
            rhs=bt_sb[:, k, :],
            start=(k == 0),
            stop=(k == KT - 1),
        )

    # ---- v^T = d*t^T + h_b^T  (feature-major, bf16 out) ------------------
    vtmp = workp.tile([P, S], f32, tag="vtmp")
    nc.vector.tensor_tensor(out=vtmp[:], in0=ps_t[:], in1=drep_sb[:], op=MUL)
    vbf = workp.tile([P, S], bf16, tag="vbf")
    nc.vector.tensor_add(out=vbf[:], in0=vtmp[:], in1=hbT_sb[:])

    # ---- transpose v -> node-major [128, 4, 128], AllGather --------------
    v_blk = workp.tile([P, KS, D], bf16, tag="vblk")
    for b in range(KS):
        tv = psTrV.tile([P, P], bf16, tag="trv")
        nc.tensor.transpose(
            out=tv[:], in_=vbf[:, b * 128 : (b + 1) * 128], identity=ident_bf[:]
        )
        nc.vector.tensor_copy(v_blk[:, b, :], tv[:])
    agin2 = dramp.tile([P, KS, D], bf16, tag="agin2")
    agout2 = dramp.tile([M * P, KS, D], bf16, tag="agout2", addr_space="Shared")
    nc.sync.dma_start(out=agin2[:], in_=v_blk[:])
    nc.gpsimd.collective_compute(
        "AllGather",
        nc_alu_bypass(),
        replica_groups=[list(range(M))],
        ins=[agin2[:].opt()],
        outs=[agout2[:].opt()],
    )
    v_node = bigp.tile([P, KT, D], bf16, tag="vn")
    for m in range(M):
        nc.sync.dma_start(
            out=v_node[:, m * KS : (m + 1) * KS, :],
            in_=agout2[m * P : (m + 1) * P, :, :],
        )

    # ---- SpMM2: out_sh^T = (A[sh,:] @ v)^T  [D, S] -----------------------
    ps_o = psO.tile([P, S], f32, tag="po")
    for k in range(KT):
        nc.tensor.matmul(
            ps_o[:],
            lhsT=v_node[:, k, :],
            rhs=at_sb[:, k, :],
            start=(k == 0),
            stop=(k == KT - 1),
        )
    o_sb = workp.tile([P, S], f32, tag="osb")
    nc.vector.tensor_copy(o_sb[:], ps_o[:])
    nc.sync.dma_start(out=outT[:], in_=o_sb[:])


def nc_alu_bypass():
    import concourse.mybir as mybir

    return mybir.AluOpType.bypass


def nc_alu_add():
    import concourse.mybir as mybir

    return mybir.AluOpType.add


def _build(split_v=False, L=1, bigb=2, dramb=2, psb=2):
    key = ("g", L, bigb, dramb, psb)
    if key in _BUILT:
        return _BUILT[key]
    import concourse.bacc as bacc
    import concourse.mybir as mybir
    import concourse.tile as tile
    from concourse.masks import make_identity

    f32 = mybir.dt.float32
    bf16 = mybir.dt.bfloat16
    fp8 = mybir.dt.float8e4

    nc = bacc.Bacc("TRN2", target_bir_lowering=False, debug=False, num_devices=M)
    xa = nc.declare_dram_parameter("xa", [P, 4, FA], bf16, isOutput=False)
    wa = nc.declare_dram_parameter("wa", [P, 4, D], bf16, isOutput=False)
    xb = nc.declare_dram_parameter("xb", [P, 4, FB], bf16, isOutput=False)
    wb = nc.declare_dram_parameter("wb", [P, 4, D], bf16, isOutput=False)
    bias = nc.declare_dram_parameter("bias", [P, 2], f32, isOutput=False)
    drep = nc.declare_dram_parameter("drep", [P, S], bf16, isOutput=False)
    bt = nc.declare_dram_parameter("bt", [P, KT, S], fp8, isOutput=False)
    at = nc.declare_dram_parameter("at", [P, KT, S], fp8, isOutput=False)
    outT = nc.declare_dram_parameter("outT", [P, S], f32, isOutput=True)

    with tile.TileContext(nc) as tc:
        ctxs = []

        def pool(name, bufs, space="SBUF"):
            p = tc.tile_pool(name=name, bufs=bufs, space=space)
            ctxs.append(p)
            return p.__enter__()

        constp = pool("const", 1)
        bigp = pool("big", bigb)
        inp = pool("in", 2)
        workp = pool("work", 2)
        psA = pool("psA", 1, "PSUM")
        psB = pool("psB", 1, "PSUM")
        psT = pool("psT", psb, "PSUM")
        psO = pool("psO", psb, "PSUM")
        psTr = pool("psTr", 1, "PSUM")
        psTrV = pool("psTrV", 1, "PSUM")
        dramp = pool("dram", dramb, "DRAM")
        try:
            ident_bf = constp.tile([P, P], bf16, tag="idbf")
            make_identity(nc, ident_bf[:])
            pools = (bigp, inp, workp, psA, psB, psT, psO, psTr, psTrV, dramp)
            tensors = (
                xa[:], wa[:], xb[:], wb[:], bias[:], drep[:], bt[:], at[:], outT[:]
            )
            for _ in range(L):
                _emit_iter(nc, tc, pools, tensors, (ident_bf,))
        finally:
            for p in reversed(ctxs):
                p.__exit__(None, None, None)
    nc.compile()
    _BUILT[key] = nc
    return nc


def _swz(a, kt):
    """[kt*128, n] row-major -> [128, kt, n] partition-major contiguous."""
    n = a.shape[1]
    return np.ascontiguousarray(a.reshape(kt, P, n).transpose(1, 0, 2))


def make_in_maps(x_a, x_b, W_a, b_a, W_b, b_b, edge_index_ab, edge_index_ba):
    bf = ml_dtypes.bfloat16
    f8 = ml_dtypes.float8_e4m3
    x_a = np.asarray(x_a, np.float32)
    x_b = np.asarray(x_b, np.float32)
    W_a = np.asarray(W_a, np.float32)
    W_b = np.asarray(W_b, np.float32)
    b_a = np.asarray(b_a, np.float32).reshape(-1)
    b_b = np.asarray(b_b, np.float32).reshape(-1)
    ea = np.asarray(edge_index_ab).astype(np.int64)
    eb = np.asarray(edge_index_ba).astype(np.int64)

    # Dense transposed adjacencies with duplicate accumulation.
    AT = (
        np.bincount(ea[1] * NA + ea[0], minlength=NA * NB)
        .reshape(NB, NA)
        .astype(np.float32)
    )  # AT[c, r] = A[r, c]
    BT = (
        np.bincount(eb[1] * NB + eb[0], minlength=NA * NB)
        .reshape(NA, NB)
        .astype(np.float32)
    )  # BT[c, r] = B[r, c]
    deg = (
        np.bincount(ea[1], minlength=NB) + np.bincount(eb[0], minlength=NB)
    ).astype(np.float32)
    d = np.where(
        deg > 0, np.float32(1.0) / np.maximum(deg, np.float32(1.0)), np.float32(0.0)
    ).astype(np.float32)

    # Edge counts are small integers; fp8e4m3 is exact for integers <= 16.
    assert AT.max() <= 16 and BT.max() <= 16, (AT.max(), BT.max())
    AT8 = AT.astype(f8)
    BT8 = BT.astype(f8)

    wa_bf = _swz(W_a, 4).astype(bf)
    wb_bf = _swz(W_b, 4).astype(bf)
    bias = np.ascontiguousarray(np.stack([b_a, b_b], axis=1))  # [128, 2]
    xaT = np.ascontiguousarray(x_a.T)  # [FA, NA]
    xbT = np.ascontiguousarray(x_b.T)  # [FB, NB]

    in_maps = []
    for m in range(M):
        sl = slice(m * S, (m + 1) * S)
        in_maps.append(
            {
                "xa": _swz(np.ascontiguousarray(xaT[:, sl]), 4).astype(bf),
                "wa": wa_bf,
                "xb": _swz(np.ascontiguousarray(xbT[:, sl]), 4).astype(bf),
                "wb": wb_bf,
                "bias": bias,
                "drep": np.ascontiguousarray(
                    np.broadcast_to(d[sl].astype(bf), (P, S))
                ),
                "bt": _swz(np.ascontiguousarray(BT8[:, sl]), KT),
                "at": _swz(np.ascontiguousarray(AT8[:, sl]), KT),
            }
        )
    return in_maps


def run(inputs, split_v=False, L=1, trace=False):
    from concourse.bass_utils import run_bass_kernel_spmd

    nc = _build(split_v=split_v, L=L)
    in_maps = make_in_maps(**inputs)
    res = run_bass_kernel_spmd(nc, in_maps, core_ids=list(range(M)), trace=trace)
    out = np.concatenate(
        [np.asarray(r["outT"]).T for r in res.results], axis=0
    )
    return out.astype(np.float32), res


def kernel(**inputs):
    out, _ = run(inputs, split_v=False, L=1, trace=False)
    return out
